# revision 18
# baseline (speedup 1.0000x reference)
"""BiRNN (tanh SimpleRNN, both directions) as a Bass/Tile kernel on 8 trn2 cores.

Problem: x [64, 512, 512] fp32; per direction W [512,512], U [512,512], b [512].
  fw:  h_t = tanh(x_t @ Wf + h_{t-1} @ Uf + bf),  ys_fw[t] = h_t
  bw:  same over time-reversed x, outputs kept in loop order.
  out[b, t, :] = concat(fw[t, b], bw[t, b])  -> [64, 512, 1024] fp32

Sharding: 8 cores = 2 directions x 4 batch groups of 16. Weights replicated
per direction; the time recurrence stays on-core (cannot be sharded).

Per-core device program (SPMD; per-core differences are data only -- bw cores
receive time-reversed x and the bw weights):
  1. xw^T precompute: psum += Wt[k,m].T @ x^T (fp16 operands, fp32 psum),
     drained by DVE tensor_scalar_add(+bias) into fp16 SBUF quarter-tiles
     xwq[j][q]: [128 h, 4 m, 16 b, 32 t].  Units are column-blocked
     (t-quarter outer) so the recurrence can start after the first four
     units; the rest streams one matmul per step into the recurrence's PE
     idle windows (x double-buffered per block from DRAM).
  2. 512 sequential steps, state kept transposed (h^T: partitions = hidden):
     psum[128, 4, 16]  = I128.T @ xw cols         (accumulation start; emitted
                                                   one step ahead so it runs
                                                   inside the ACT latency)
     psum[:, m, :]    += Ut[k,m].T @ ht_{t-1}[:, k, :]   (16 LDW+MM pairs)
     ht_t              = tanh(psum)               (ONE activation, psum ->
                                                   small contiguous SBUF tile)
     outb cols         = ht_t                     (DVE copy, off critical path)
  3. Output half-tiles [128, 64, 4, 16] fp16 DMA out as soon as filled.

Host: pre-transposes/casts inputs per core, gathers [4,128,128,4,16] fp16
outputs, reassembles the [64, 512, 1024] fp32 result.
"""

import numpy as np

B, T, F, H = 64, 512, 512, 512
NCORES = 8
NGROUP = 4            # batch groups
BL = B // NGROUP      # 16 batch rows per core
KC = F // 128         # 4 contraction chunks
MC = H // 128         # 4 output chunks
TQ = 32               # precompute column-block width

_PROGRAM_CACHE = {}


def _build_program(steps=T):
    import concourse.mybir as mybir
    import concourse.tile as tile
    from concourse import bacc

    f16 = mybir.dt.float16
    f32 = mybir.dt.float32
    Tanh = mybir.ActivationFunctionType.Tanh
    nblocks = steps // 128
    NQ = 128 // TQ  # quarters per block

    nc = bacc.Bacc("TRN2", target_bir_lowering=False, debug=False)

    xTb = nc.dram_tensor(
        "xTb", [KC, nblocks, 128, BL, 128], f16, kind="ExternalInput"
    ).ap()
    Wt = nc.dram_tensor("Wt", [KC, MC, 128, 128], f16, kind="ExternalInput").ap()
    Ut = nc.dram_tensor("Ut", [KC, MC, 128, 128], f16, kind="ExternalInput").ap()
    bT = nc.dram_tensor("bT", [MC, 128, 1], f32, kind="ExternalInput").ap()
    eye = nc.dram_tensor("eye", [128, 128], f16, kind="ExternalInput").ap()
    ys = nc.dram_tensor(
        "ys", [nblocks, 128, 128, MC, BL], f16, kind="ExternalOutput"
    ).ap()

    with tile.TileContext(nc) as tc:
        with (
            tc.tile_pool(name="weights", bufs=1) as wpool,
            tc.tile_pool(name="xstage", bufs=2) as xpool,
            tc.tile_pool(name="xwbuf", bufs=1) as xwpool,
            tc.tile_pool(name="outbuf", bufs=1) as outpool,
            tc.tile_pool(name="htbuf", bufs=4) as htpool,
            tc.tile_pool(name="pcpsum", bufs=2, space="PSUM") as pcpool,
            tc.tile_pool(name="rpsum", bufs=3, space="PSUM") as rpool,
        ):
            def x_dma(j):
                # one batched DMA per time block: [128, (k, b, tl)]
                xs = xpool.tile([128, KC, BL, 128], f16, tag="xs", name=f"xs_{j}")
                nc.sync.dma_start(xs[:], xTb[:, j].rearrange("k p b t -> p k b t"))
                return xs

            # x block 0 first, split across two queues (sync + gpsimd) so the
            # precompute prologue unblocks in half the transfer time; weights
            # go on a third queue (vector) in parallel.
            xs_cur = xpool.tile([128, KC, BL, 128], f16, tag="xs", name="xs_0")
            nc.sync.dma_start(
                xs_cur[:, 0:2], xTb[0:2, 0].rearrange("k p b t -> p k b t")
            )
            nc.gpsimd.dma_start(
                xs_cur[:, 2:4], xTb[2:4, 0].rearrange("k p b t -> p k b t")
            )
            # batched weight loads: one DMA each for W and U, [128, (k, m, col)]
            W_all = wpool.tile([128, KC, MC, 128], f16, tag="W_all", name="W_all")
            nc.scalar.dma_start(W_all[:], Wt.rearrange("k m p c -> p k m c"))
            W_sb = [[W_all[:, k, m, :] for m in range(MC)] for k in range(KC)]
            U_all = wpool.tile([128, KC, MC, 128], f16, tag="U_all", name="U_all")
            nc.scalar.dma_start(U_all[:], Ut.rearrange("k m p c -> p k m c"))
            U_sb = [[U_all[:, k, m, :] for m in range(MC)] for k in range(KC)]
            b_all = wpool.tile([128, MC], f32, tag="b_all", name="b_all")
            nc.scalar.dma_start(b_all[:], bT.rearrange("m p o -> p (m o)"))
            b_sb = [b_all[:, m : m + 1] for m in range(MC)]
            eye_sb = wpool.tile([128, 128], f16, tag="eye", name="eye_sb")
            nc.scalar.dma_start(eye_sb[:], eye[:])

            # xw^T quarter-tiles (pc-written, injection-read)
            xwq = [
                [
                    xwpool.tile(
                        [128, MC, BL, TQ], f16, tag=f"xw{j}_{q}", name=f"xw{j}_{q}"
                    )
                    for q in range(NQ)
                ]
                for j in range(nblocks)
            ]
            # output quarter-tiles (DVE-written, DMA-read)
            outb = [
                [
                    outpool.tile(
                        [128, 32, MC, BL], f16, tag=f"out{j}_{h}", name=f"outb{j}_{h}"
                    )
                    for h in range(4)
                ]
                for j in range(nblocks)
            ]

            def pc_unit_mm(xs_tile, q, m, k, ps):
                nc.tensor.matmul(
                    ps[:],
                    W_sb[k][m],
                    xs_tile[:, k, :, TQ * q : TQ * q + TQ],
                    start=(k == 0),
                    stop=(k == KC - 1),
                )

            def pc_unit_drain(j, q, m, ps):
                # += bias while downcasting to fp16
                nc.vector.tensor_scalar_add(
                    xwq[j][q][:, m, :, :], ps[:], b_sb[m]
                )

            # t-quarter outer so the first columns are ready after 4 units
            pc_units = [(q, m) for q in range(NQ) for m in range(MC)]

            # Prologue: precompute only quarter 0 of block 0; the rest of
            # block 0 streams into the first steps so the PE queue stays short
            # ahead of the recurrence.
            for (q, m) in pc_units[:MC]:
                ps = pcpool.tile([128, BL, TQ], f32, tag="pc", name=f"pc0_{q}_{m}")
                for k in range(KC):
                    pc_unit_mm(xs_cur, q, m, k, ps)
                pc_unit_drain(0, q, m, ps)

            # Streamed precompute: one matmul per step. Work list per step
            # window: block 0 steps 0..47 finish block 0 (12 units); block 0
            # steps 48..111 do block 1; block j>=1 steps 8..71 do block j+1.
            pc_state = {"xs": {0: xs_cur}}

            def pc_mm_seq(jtgt, units, s):
                u, k = divmod(s, 4)
                q, m = units[u]
                if k == 0:
                    pc_state["ps"] = pcpool.tile(
                        [128, BL, TQ], f32, tag="pc", name=f"pc{jtgt}_{q}_{m}"
                    )
                pc_unit_mm(pc_state["xs"][jtgt], q, m, k, pc_state["ps"])
                if k == KC - 1:
                    pc_unit_drain(jtgt, q, m, pc_state["ps"])

            def pc_step(j, s):
                if j == 0:
                    if s < 48:
                        pc_mm_seq(0, pc_units[MC:], s)
                    elif s == 48 and nblocks > 1:
                        pc_state["xs"][1] = x_dma(1)
                    elif 56 <= s < 56 + 64 and nblocks > 1:
                        pc_mm_seq(1, pc_units, s - 56)
                elif j + 1 < nblocks:
                    if s == 0:
                        pc_state["xs"][j + 1] = x_dma(j + 1)
                    elif 8 <= s < 8 + 64:
                        pc_mm_seq(j + 1, pc_units, s - 8)

            # Recurrence. The xw injection for step t+1 is emitted BEFORE step
            # t's activation so (a) the PE executes it inside the activation
            # latency window and (b) Tile's cross-engine wait for ht(t) lands
            # on the first U matmul, not the injection.
            def inject_xw(t):
                j, tl = divmod(t, 128)
                ps = rpool.tile([128, MC, BL], f32, tag="ps", name=f"ps_{t}")
                nc.tensor.matmul(
                    ps[:],
                    eye_sb[:],
                    xwq[j][tl // TQ][:, :, :, tl % TQ],
                    start=True,
                    stop=False,
                    skip_group_check=True,
                )
                return ps

            ht_prev = None
            ps_next = None
            for t in range(steps):
                j, tl = divmod(t, 128)
                ht = htpool.tile([128, MC, BL], f16, tag="ht", name=f"ht{t}")
                if t == 0:
                    ps_next = inject_xw(1)
                    nc.scalar.activation(ht[:], xwq[0][0][:, :, :, 0], Tanh)
                else:
                    ps_t = ps_next
                    for k in range(KC):
                        hprev = ht_prev[:, k, :]
                        for m in range(MC):
                            nc.tensor.matmul(
                                ps_t[:, m, :],
                                U_sb[k][m],
                                hprev,
                                start=False,
                                stop=(k == KC - 1),
                                skip_group_check=True,
                            )
                    if t + 1 < steps:
                        ps_next = inject_xw(t + 1)
                    nc.scalar.activation(ht[:], ps_t[:], Tanh)
                nc.vector.tensor_copy(outb[j][tl // 32][:, tl % 32, :, :], ht[:])
                ht_prev = ht
                pc_step(j, tl)
                if tl % 32 == 31:
                    h = tl // 32
                    nc.sync.dma_start(
                        ys[j][:, 32 * h : 32 * h + 32], outb[j][h][:]
                    )

    nc.compile()
    return nc


def get_program(steps=T):
    if steps not in _PROGRAM_CACHE:
        _PROGRAM_CACHE[steps] = _build_program(steps)
    return _PROGRAM_CACHE[steps]


def make_in_maps(x, Wf, Uf, bf, Wb, Ub, bb, steps=T):
    """Per-core input dicts. Core c: direction c//4 (0 fw, 1 bw), batch group c%4."""
    x = np.asarray(x, dtype=np.float32)
    eye = np.eye(128, dtype=np.float16)
    nblocks = steps // 128
    in_maps = []
    for c in range(NCORES):
        d, g = divmod(c, NGROUP)
        xs = x[g * BL : (g + 1) * BL, :steps]
        if d == 1:
            xs = xs[:, ::-1]
        # xTb[k, j, p, b, tl] = xs[b, 128j + tl, 128k + p]
        xTc = xs.transpose(2, 0, 1).astype(np.float16).reshape(KC, 128, BL, steps)
        xTbc = np.ascontiguousarray(
            xTc.reshape(KC, 128, BL, nblocks, 128).transpose(0, 3, 1, 2, 4)
        )
        W, U, bvec = (Wf, Uf, bf) if d == 0 else (Wb, Ub, bb)
        Wtc = np.ascontiguousarray(
            np.asarray(W, np.float32).reshape(KC, 128, MC, 128).transpose(0, 2, 1, 3)
        ).astype(np.float16)
        Utc = np.ascontiguousarray(
            np.asarray(U, np.float32).reshape(KC, 128, MC, 128).transpose(0, 2, 1, 3)
        ).astype(np.float16)
        bTc = np.asarray(bvec, np.float32).reshape(MC, 128, 1)
        in_maps.append({"xTb": xTbc, "Wt": Wtc, "Ut": Utc, "bT": bTc, "eye": eye})
    return in_maps


def assemble_output(per_core_ys, steps=T):
    out = np.empty((B, steps, 2 * H), dtype=np.float32)
    for c in range(NCORES):
        d, g = divmod(c, NGROUP)
        ysc = np.asarray(per_core_ys[c])  # [nblocks, 128, 128, MC, BL] fp16
        # out[b, 128j+tl, 128m+p] = ys[j, p, tl, m, b]
        y = ysc.transpose(4, 0, 2, 3, 1).reshape(BL, steps, H).astype(np.float32)
        out[g * BL : (g + 1) * BL, :, d * H : (d + 1) * H] = y
    return out


def kernel(**inputs):
    nc = get_program(T)
    in_maps = make_in_maps(
        inputs["x"], inputs["Wf"], inputs["Uf"], inputs["bf"],
        inputs["Wb"], inputs["Ub"], inputs["bb"],
    )
    from concourse.bass_utils import run_bass_kernel_spmd

    res = run_bass_kernel_spmd(nc, in_maps, list(range(NCORES)))
    return assemble_output([res.results[c]["ys"] for c in range(NCORES)])


# revision 19
# speedup vs baseline: 1.0048x; 1.0048x over previous
"""BiRNN (tanh SimpleRNN, both directions) as a Bass/Tile kernel on 8 trn2 cores.

Problem: x [64, 512, 512] fp32; per direction W [512,512], U [512,512], b [512].
  fw:  h_t = tanh(x_t @ Wf + h_{t-1} @ Uf + bf),  ys_fw[t] = h_t
  bw:  same over time-reversed x, outputs kept in loop order.
  out[b, t, :] = concat(fw[t, b], bw[t, b])  -> [64, 512, 1024] fp32

Sharding: 8 cores = 2 directions x 4 batch groups of 16. Weights replicated
per direction; the time recurrence stays on-core (cannot be sharded).

Per-core device program (SPMD; per-core differences are data only -- bw cores
receive time-reversed x and the bw weights):
  1. xw^T precompute: psum += Wt[k,m].T @ x^T (fp16 operands, fp32 psum),
     drained by DVE tensor_scalar_add(+bias) into fp16 SBUF quarter-tiles
     xwq[j][q]: [128 h, 4 m, 16 b, 32 t].  Units are column-blocked
     (t-quarter outer) so the recurrence can start after the first four
     units; the rest streams one matmul per step into the recurrence's PE
     idle windows (x double-buffered per block from DRAM).
  2. 512 sequential steps, state kept transposed (h^T: partitions = hidden):
     psum[128, 4, 16]  = I128.T @ xw cols         (accumulation start; emitted
                                                   one step ahead so it runs
                                                   inside the ACT latency)
     psum[:, m, :]    += Ut[k,m].T @ ht_{t-1}[:, k, :]   (16 LDW+MM pairs)
     ht_t              = tanh(psum)               (ONE activation, psum ->
                                                   small contiguous SBUF tile)
     outb cols         = ht_t                     (DVE copy, off critical path)
  3. Output half-tiles [128, 64, 4, 16] fp16 DMA out as soon as filled.

Host: pre-transposes/casts inputs per core, gathers [4,128,128,4,16] fp16
outputs, reassembles the [64, 512, 1024] fp32 result.
"""

import numpy as np

B, T, F, H = 64, 512, 512, 512
NCORES = 8
NGROUP = 4            # batch groups
BL = B // NGROUP      # 16 batch rows per core
KC = F // 128         # 4 contraction chunks
MC = H // 128         # 4 output chunks
TQ = 32               # precompute column-block width

_PROGRAM_CACHE = {}


def _build_program(steps=T):
    import concourse.mybir as mybir
    import concourse.tile as tile
    from concourse import bacc

    f16 = mybir.dt.float16
    f32 = mybir.dt.float32
    Tanh = mybir.ActivationFunctionType.Tanh
    nblocks = steps // 128
    NQ = 128 // TQ  # quarters per block

    nc = bacc.Bacc("TRN2", target_bir_lowering=False, debug=False)

    xTb = nc.dram_tensor(
        "xTb", [KC, nblocks, 128, BL, 128], f16, kind="ExternalInput"
    ).ap()
    Wt = nc.dram_tensor("Wt", [KC, MC, 128, 128], f16, kind="ExternalInput").ap()
    Ut = nc.dram_tensor("Ut", [KC, MC, 128, 128], f16, kind="ExternalInput").ap()
    bT = nc.dram_tensor("bT", [MC, 128, 1], f32, kind="ExternalInput").ap()
    eye = nc.dram_tensor("eye", [128, 128], f16, kind="ExternalInput").ap()
    ys = nc.dram_tensor(
        "ys", [nblocks, 128, 128, MC, BL], f16, kind="ExternalOutput"
    ).ap()

    with tile.TileContext(nc) as tc:
        with (
            tc.tile_pool(name="weights", bufs=1) as wpool,
            tc.tile_pool(name="xstage", bufs=2) as xpool,
            tc.tile_pool(name="xwbuf", bufs=1) as xwpool,
            tc.tile_pool(name="outbuf", bufs=1) as outpool,
            tc.tile_pool(name="htbuf", bufs=4) as htpool,
            tc.tile_pool(name="pcpsum", bufs=2, space="PSUM") as pcpool,
            tc.tile_pool(name="rpsum", bufs=3, space="PSUM") as rpool,
        ):
            def x_dma(j):
                # one batched DMA per time block: [128, (k, b, tl)]
                xs = xpool.tile([128, KC, BL, 128], f16, tag="xs", name=f"xs_{j}")
                nc.sync.dma_start(xs[:], xTb[:, j].rearrange("k p b t -> p k b t"))
                return xs

            # x block 0 first, split across two HWDGE queues (sync + scalar)
            # so the precompute prologue unblocks in half the transfer time.
            xs_cur = xpool.tile([128, KC, BL, 128], f16, tag="xs", name="xs_0")
            nc.sync.dma_start(
                xs_cur[:, 0:2], xTb[0:2, 0].rearrange("k p b t -> p k b t")
            )
            nc.scalar.dma_start(
                xs_cur[:, 2:4], xTb[2:4, 0].rearrange("k p b t -> p k b t")
            )
            # batched weight loads: one DMA each for W and U, [128, (k, m, col)]
            W_all = wpool.tile([128, KC, MC, 128], f16, tag="W_all", name="W_all")
            nc.sync.dma_start(W_all[:], Wt.rearrange("k m p c -> p k m c"))
            W_sb = [[W_all[:, k, m, :] for m in range(MC)] for k in range(KC)]
            U_all = wpool.tile([128, KC, MC, 128], f16, tag="U_all", name="U_all")
            nc.scalar.dma_start(U_all[:], Ut.rearrange("k m p c -> p k m c"))
            U_sb = [[U_all[:, k, m, :] for m in range(MC)] for k in range(KC)]
            b_all = wpool.tile([128, MC], f32, tag="b_all", name="b_all")
            nc.scalar.dma_start(b_all[:], bT.rearrange("m p o -> p (m o)"))
            b_sb = [b_all[:, m : m + 1] for m in range(MC)]
            eye_sb = wpool.tile([128, 128], f16, tag="eye", name="eye_sb")
            nc.scalar.dma_start(eye_sb[:], eye[:])

            # xw^T quarter-tiles (pc-written, injection-read)
            xwq = [
                [
                    xwpool.tile(
                        [128, MC, BL, TQ], f16, tag=f"xw{j}_{q}", name=f"xw{j}_{q}"
                    )
                    for q in range(NQ)
                ]
                for j in range(nblocks)
            ]
            # output quarter-tiles (DVE-written, DMA-read)
            outb = [
                [
                    outpool.tile(
                        [128, 32, MC, BL], f16, tag=f"out{j}_{h}", name=f"outb{j}_{h}"
                    )
                    for h in range(4)
                ]
                for j in range(nblocks)
            ]

            def pc_unit_mm(xs_tile, q, m, k, ps):
                nc.tensor.matmul(
                    ps[:],
                    W_sb[k][m],
                    xs_tile[:, k, :, TQ * q : TQ * q + TQ],
                    start=(k == 0),
                    stop=(k == KC - 1),
                )

            def pc_unit_drain(j, q, m, ps):
                # += bias while downcasting to fp16
                nc.vector.tensor_scalar_add(
                    xwq[j][q][:, m, :, :], ps[:], b_sb[m]
                )

            # t-quarter outer so the first columns are ready after 4 units
            pc_units = [(q, m) for q in range(NQ) for m in range(MC)]

            # Prologue: precompute only quarter 0 of block 0; the rest of
            # block 0 streams into the first steps so the PE queue stays short
            # ahead of the recurrence.
            for (q, m) in pc_units[:MC]:
                ps = pcpool.tile([128, BL, TQ], f32, tag="pc", name=f"pc0_{q}_{m}")
                for k in range(KC):
                    pc_unit_mm(xs_cur, q, m, k, ps)
                pc_unit_drain(0, q, m, ps)

            # Streamed precompute: one matmul per step. Work list per step
            # window: block 0 steps 0..47 finish block 0 (12 units); block 0
            # steps 48..111 do block 1; block j>=1 steps 8..71 do block j+1.
            pc_state = {"xs": {0: xs_cur}}

            def pc_mm_seq(jtgt, units, s):
                u, k = divmod(s, 4)
                q, m = units[u]
                if k == 0:
                    pc_state["ps"] = pcpool.tile(
                        [128, BL, TQ], f32, tag="pc", name=f"pc{jtgt}_{q}_{m}"
                    )
                pc_unit_mm(pc_state["xs"][jtgt], q, m, k, pc_state["ps"])
                if k == KC - 1:
                    pc_unit_drain(jtgt, q, m, pc_state["ps"])

            def pc_step(j, s):
                if j == 0:
                    if s < 48:
                        pc_mm_seq(0, pc_units[MC:], s)
                    elif s == 48 and nblocks > 1:
                        pc_state["xs"][1] = x_dma(1)
                    elif 56 <= s < 56 + 64 and nblocks > 1:
                        pc_mm_seq(1, pc_units, s - 56)
                elif j + 1 < nblocks:
                    if s == 0:
                        pc_state["xs"][j + 1] = x_dma(j + 1)
                    elif 8 <= s < 8 + 64:
                        pc_mm_seq(j + 1, pc_units, s - 8)

            # Recurrence. The xw injection for step t+1 is emitted BEFORE step
            # t's activation so (a) the PE executes it inside the activation
            # latency window and (b) Tile's cross-engine wait for ht(t) lands
            # on the first U matmul, not the injection.
            def inject_xw(t):
                j, tl = divmod(t, 128)
                ps = rpool.tile([128, MC, BL], f32, tag="ps", name=f"ps_{t}")
                nc.tensor.matmul(
                    ps[:],
                    eye_sb[:],
                    xwq[j][tl // TQ][:, :, :, tl % TQ],
                    start=True,
                    stop=False,
                    skip_group_check=True,
                )
                return ps

            ht_prev = None
            ps_next = None
            for t in range(steps):
                j, tl = divmod(t, 128)
                ht = htpool.tile([128, MC, BL], f16, tag="ht", name=f"ht{t}")
                if t == 0:
                    ps_next = inject_xw(1)
                    nc.scalar.activation(ht[:], xwq[0][0][:, :, :, 0], Tanh)
                else:
                    ps_t = ps_next
                    for k in range(KC):
                        hprev = ht_prev[:, k, :]
                        for m in range(MC):
                            nc.tensor.matmul(
                                ps_t[:, m, :],
                                U_sb[k][m],
                                hprev,
                                start=False,
                                stop=(k == KC - 1),
                                skip_group_check=True,
                            )
                    if t + 1 < steps:
                        ps_next = inject_xw(t + 1)
                    nc.scalar.activation(ht[:], ps_t[:], Tanh)
                nc.vector.tensor_copy(outb[j][tl // 32][:, tl % 32, :, :], ht[:])
                ht_prev = ht
                pc_step(j, tl)
                if tl % 32 == 31:
                    h = tl // 32
                    nc.sync.dma_start(
                        ys[j][:, 32 * h : 32 * h + 32], outb[j][h][:]
                    )

    nc.compile()
    return nc


def get_program(steps=T):
    if steps not in _PROGRAM_CACHE:
        _PROGRAM_CACHE[steps] = _build_program(steps)
    return _PROGRAM_CACHE[steps]


def make_in_maps(x, Wf, Uf, bf, Wb, Ub, bb, steps=T):
    """Per-core input dicts. Core c: direction c//4 (0 fw, 1 bw), batch group c%4."""
    x = np.asarray(x, dtype=np.float32)
    eye = np.eye(128, dtype=np.float16)
    nblocks = steps // 128
    in_maps = []
    for c in range(NCORES):
        d, g = divmod(c, NGROUP)
        xs = x[g * BL : (g + 1) * BL, :steps]
        if d == 1:
            xs = xs[:, ::-1]
        # xTb[k, j, p, b, tl] = xs[b, 128j + tl, 128k + p]
        xTc = xs.transpose(2, 0, 1).astype(np.float16).reshape(KC, 128, BL, steps)
        xTbc = np.ascontiguousarray(
            xTc.reshape(KC, 128, BL, nblocks, 128).transpose(0, 3, 1, 2, 4)
        )
        W, U, bvec = (Wf, Uf, bf) if d == 0 else (Wb, Ub, bb)
        Wtc = np.ascontiguousarray(
            np.asarray(W, np.float32).reshape(KC, 128, MC, 128).transpose(0, 2, 1, 3)
        ).astype(np.float16)
        Utc = np.ascontiguousarray(
            np.asarray(U, np.float32).reshape(KC, 128, MC, 128).transpose(0, 2, 1, 3)
        ).astype(np.float16)
        bTc = np.asarray(bvec, np.float32).reshape(MC, 128, 1)
        in_maps.append({"xTb": xTbc, "Wt": Wtc, "Ut": Utc, "bT": bTc, "eye": eye})
    return in_maps


def assemble_output(per_core_ys, steps=T):
    out = np.empty((B, steps, 2 * H), dtype=np.float32)
    for c in range(NCORES):
        d, g = divmod(c, NGROUP)
        ysc = np.asarray(per_core_ys[c])  # [nblocks, 128, 128, MC, BL] fp16
        # out[b, 128j+tl, 128m+p] = ys[j, p, tl, m, b]
        y = ysc.transpose(4, 0, 2, 3, 1).reshape(BL, steps, H).astype(np.float32)
        out[g * BL : (g + 1) * BL, :, d * H : (d + 1) * H] = y
    return out


def kernel(**inputs):
    nc = get_program(T)
    in_maps = make_in_maps(
        inputs["x"], inputs["Wf"], inputs["Uf"], inputs["bf"],
        inputs["Wb"], inputs["Ub"], inputs["bb"],
    )
    from concourse.bass_utils import run_bass_kernel_spmd

    res = run_bass_kernel_spmd(nc, in_maps, list(range(NCORES)))
    return assemble_output([res.results[c]["ys"] for c in range(NCORES)])


# revision 20
# speedup vs baseline: 1.0163x; 1.0115x over previous
"""BiRNN (tanh SimpleRNN, both directions) as a Bass/Tile kernel on 8 trn2 cores.

Problem: x [64, 512, 512] fp32; per direction W [512,512], U [512,512], b [512].
  fw:  h_t = tanh(x_t @ Wf + h_{t-1} @ Uf + bf),  ys_fw[t] = h_t
  bw:  same over time-reversed x, outputs kept in loop order.
  out[b, t, :] = concat(fw[t, b], bw[t, b])  -> [64, 512, 1024] fp32

Sharding: 8 cores = 2 directions x 4 batch groups of 16. Weights replicated
per direction; the time recurrence stays on-core (cannot be sharded).

Per-core device program (SPMD; per-core differences are data only -- bw cores
receive time-reversed x and the bw weights):
  1. xw^T precompute: psum += Wt[k,m].T @ x^T (fp16 operands, fp32 psum),
     drained by DVE tensor_scalar_add(+bias) into fp16 SBUF quarter-tiles
     xwq[j][q]: [128 h, 4 m, 16 b, 32 t].  Units are column-blocked
     (t-quarter outer) so the recurrence can start after the first four
     units; the rest streams one matmul per step into the recurrence's PE
     idle windows (x double-buffered per block from DRAM).
  2. 512 sequential steps, state kept transposed (h^T: partitions = hidden):
     psum[128, 4, 16]  = I128.T @ xw cols         (accumulation start; emitted
                                                   one step ahead so it runs
                                                   inside the ACT latency)
     psum[:, m, :]    += Ut[k,m].T @ ht_{t-1}[:, k, :]   (16 LDW+MM pairs)
     ht_t              = tanh(psum)               (ONE activation, psum ->
                                                   small contiguous SBUF tile)
     outb cols         = ht_t                     (DVE copy, off critical path)
  3. Output half-tiles [128, 64, 4, 16] fp16 DMA out as soon as filled.

Host: pre-transposes/casts inputs per core, gathers [4,128,128,4,16] fp16
outputs, reassembles the [64, 512, 1024] fp32 result.
"""

import numpy as np

B, T, F, H = 64, 512, 512, 512
NCORES = 8
NGROUP = 4            # batch groups
BL = B // NGROUP      # 16 batch rows per core
KC = F // 128         # 4 contraction chunks
MC = H // 128         # 4 output chunks
TQ = 32               # precompute column-block width

_PROGRAM_CACHE = {}


def _build_program(steps=T):
    import concourse.mybir as mybir
    import concourse.tile as tile
    from concourse import bacc

    f16 = mybir.dt.float16
    f32 = mybir.dt.float32
    Tanh = mybir.ActivationFunctionType.Tanh
    nblocks = steps // 128
    NQ = 128 // TQ  # quarters per block

    nc = bacc.Bacc("TRN2", target_bir_lowering=False, debug=False)

    xTb = nc.dram_tensor(
        "xTb", [KC, nblocks, 128, BL, 128], f16, kind="ExternalInput"
    ).ap()
    Wt = nc.dram_tensor("Wt", [KC, MC, 128, 128], f16, kind="ExternalInput").ap()
    Ut = nc.dram_tensor("Ut", [KC, MC, 128, 128], f16, kind="ExternalInput").ap()
    bT = nc.dram_tensor("bT", [MC, 128, 1], f32, kind="ExternalInput").ap()
    eye = nc.dram_tensor("eye", [128, 128], f16, kind="ExternalInput").ap()
    ys = nc.dram_tensor(
        "ys", [nblocks, 128, 128, MC, BL], f16, kind="ExternalOutput"
    ).ap()

    with tile.TileContext(nc) as tc:
        with (
            tc.tile_pool(name="weights", bufs=1) as wpool,
            tc.tile_pool(name="xstage", bufs=2) as xpool,
            tc.tile_pool(name="xwbuf", bufs=1) as xwpool,
            tc.tile_pool(name="outbuf", bufs=1) as outpool,
            tc.tile_pool(name="htbuf", bufs=4) as htpool,
            tc.tile_pool(name="pcpsum", bufs=2, space="PSUM") as pcpool,
            tc.tile_pool(name="rpsum", bufs=3, space="PSUM") as rpool,
        ):
            def x_dma(j):
                # one batched DMA per time block: [128, (k, b, tl)]
                xs = xpool.tile([128, KC, BL, 128], f16, tag="xs", name=f"xs_{j}")
                nc.sync.dma_start(xs[:], xTb[:, j].rearrange("k p b t -> p k b t"))
                return xs

            # x block 0 first so the precompute prologue unblocks earliest
            xs_cur = x_dma(0)
            # batched weight loads: one DMA each for W and U, [128, (k, m, col)]
            W_all = wpool.tile([128, KC, MC, 128], f16, tag="W_all", name="W_all")
            nc.sync.dma_start(W_all[:], Wt.rearrange("k m p c -> p k m c"))
            W_sb = [[W_all[:, k, m, :] for m in range(MC)] for k in range(KC)]
            b_all = wpool.tile([128, MC], f32, tag="b_all", name="b_all")
            nc.sync.dma_start(b_all[:], bT.rearrange("m p o -> p (m o)"))
            b_sb = [b_all[:, m : m + 1] for m in range(MC)]
            eye_sb = wpool.tile([128, 128], f16, tag="eye", name="eye_sb")
            nc.sync.dma_start(eye_sb[:], eye[:])
            U_all = wpool.tile([128, KC, MC, 128], f16, tag="U_all", name="U_all")
            nc.sync.dma_start(U_all[:], Ut.rearrange("k m p c -> p k m c"))
            U_sb = [[U_all[:, k, m, :] for m in range(MC)] for k in range(KC)]

            # xw^T quarter-tiles (pc-written, injection-read)
            xwq = [
                [
                    xwpool.tile(
                        [128, MC, BL, TQ], f16, tag=f"xw{j}_{q}", name=f"xw{j}_{q}"
                    )
                    for q in range(NQ)
                ]
                for j in range(nblocks)
            ]
            # output quarter-tiles (DVE-written, DMA-read)
            outb = [
                [
                    outpool.tile(
                        [128, 32, MC, BL], f16, tag=f"out{j}_{h}", name=f"outb{j}_{h}"
                    )
                    for h in range(4)
                ]
                for j in range(nblocks)
            ]

            def pc_unit_mm(xs_tile, q, m, k, ps):
                nc.tensor.matmul(
                    ps[:],
                    W_sb[k][m],
                    xs_tile[:, k, :, TQ * q : TQ * q + TQ],
                    start=(k == 0),
                    stop=(k == KC - 1),
                )

            def pc_unit_drain(j, q, m, ps):
                # += bias while downcasting to fp16
                nc.vector.tensor_scalar_add(
                    xwq[j][q][:, m, :, :], ps[:], b_sb[m]
                )

            # t-quarter outer so the first columns are ready after 4 units
            pc_units = [(q, m) for q in range(NQ) for m in range(MC)]

            # Prologue: precompute only quarter 0 of block 0; the rest of
            # block 0 streams into the first steps so the PE queue stays short
            # ahead of the recurrence.
            for (q, m) in pc_units[:MC]:
                ps = pcpool.tile([128, BL, TQ], f32, tag="pc", name=f"pc0_{q}_{m}")
                for k in range(KC):
                    pc_unit_mm(xs_cur, q, m, k, ps)
                pc_unit_drain(0, q, m, ps)

            # Streamed precompute: one matmul per step. Work list per step
            # window: block 0 steps 0..47 finish block 0 (12 units); block 0
            # steps 48..111 do block 1; block j>=1 steps 8..71 do block j+1.
            pc_state = {"xs": {0: xs_cur}}

            def pc_mm_seq(jtgt, units, s):
                u, k = divmod(s, 4)
                q, m = units[u]
                if k == 0:
                    pc_state["ps"] = pcpool.tile(
                        [128, BL, TQ], f32, tag="pc", name=f"pc{jtgt}_{q}_{m}"
                    )
                pc_unit_mm(pc_state["xs"][jtgt], q, m, k, pc_state["ps"])
                if k == KC - 1:
                    pc_unit_drain(jtgt, q, m, pc_state["ps"])

            def pc_step(j, s):
                if j == 0:
                    if s < 48:
                        pc_mm_seq(0, pc_units[MC:], s)
                    elif s == 48 and nblocks > 1:
                        pc_state["xs"][1] = x_dma(1)
                    elif 56 <= s < 56 + 64 and nblocks > 1:
                        pc_mm_seq(1, pc_units, s - 56)
                elif j + 1 < nblocks:
                    if s == 0:
                        pc_state["xs"][j + 1] = x_dma(j + 1)
                    elif 8 <= s < 8 + 64:
                        pc_mm_seq(j + 1, pc_units, s - 8)

            # Recurrence. The xw injection for step t+1 is emitted BEFORE step
            # t's activation so (a) the PE executes it inside the activation
            # latency window and (b) Tile's cross-engine wait for ht(t) lands
            # on the first U matmul, not the injection.
            def inject_xw(t):
                j, tl = divmod(t, 128)
                ps = rpool.tile([128, MC, BL], f32, tag="ps", name=f"ps_{t}")
                nc.tensor.matmul(
                    ps[:],
                    eye_sb[:],
                    xwq[j][tl // TQ][:, :, :, tl % TQ],
                    start=True,
                    stop=False,
                    skip_group_check=True,
                )
                return ps

            ht_prev = None
            ps_next = None
            for t in range(steps):
                j, tl = divmod(t, 128)
                ht = htpool.tile([128, MC, BL], f16, tag="ht", name=f"ht{t}")
                if t == 0:
                    ps_next = inject_xw(1)
                    nc.scalar.activation(ht[:], xwq[0][0][:, :, :, 0], Tanh)
                else:
                    ps_t = ps_next
                    for k in range(KC):
                        hprev = ht_prev[:, k, :]
                        for m in range(MC):
                            nc.tensor.matmul(
                                ps_t[:, m, :],
                                U_sb[k][m],
                                hprev,
                                start=False,
                                stop=(k == KC - 1),
                                skip_group_check=True,
                            )
                    if t + 1 < steps:
                        ps_next = inject_xw(t + 1)
                    nc.scalar.activation(ht[:], ps_t[:], Tanh)
                nc.vector.tensor_copy(outb[j][tl // 32][:, tl % 32, :, :], ht[:])
                ht_prev = ht
                pc_step(j, tl)
                if tl % 32 == 31:
                    h = tl // 32
                    nc.sync.dma_start(
                        ys[j][:, 32 * h : 32 * h + 32], outb[j][h][:]
                    )

    nc.compile()
    return nc


def get_program(steps=T):
    if steps not in _PROGRAM_CACHE:
        _PROGRAM_CACHE[steps] = _build_program(steps)
    return _PROGRAM_CACHE[steps]


def make_in_maps(x, Wf, Uf, bf, Wb, Ub, bb, steps=T):
    """Per-core input dicts. Core c: direction c//4 (0 fw, 1 bw), batch group c%4."""
    x = np.asarray(x, dtype=np.float32)
    eye = np.eye(128, dtype=np.float16)
    nblocks = steps // 128
    in_maps = []
    for c in range(NCORES):
        d, g = divmod(c, NGROUP)
        xs = x[g * BL : (g + 1) * BL, :steps]
        if d == 1:
            xs = xs[:, ::-1]
        # xTb[k, j, p, b, tl] = xs[b, 128j + tl, 128k + p]
        xTc = xs.transpose(2, 0, 1).astype(np.float16).reshape(KC, 128, BL, steps)
        xTbc = np.ascontiguousarray(
            xTc.reshape(KC, 128, BL, nblocks, 128).transpose(0, 3, 1, 2, 4)
        )
        W, U, bvec = (Wf, Uf, bf) if d == 0 else (Wb, Ub, bb)
        Wtc = np.ascontiguousarray(
            np.asarray(W, np.float32).reshape(KC, 128, MC, 128).transpose(0, 2, 1, 3)
        ).astype(np.float16)
        Utc = np.ascontiguousarray(
            np.asarray(U, np.float32).reshape(KC, 128, MC, 128).transpose(0, 2, 1, 3)
        ).astype(np.float16)
        bTc = np.asarray(bvec, np.float32).reshape(MC, 128, 1)
        in_maps.append({"xTb": xTbc, "Wt": Wtc, "Ut": Utc, "bT": bTc, "eye": eye})
    return in_maps


def assemble_output(per_core_ys, steps=T):
    out = np.empty((B, steps, 2 * H), dtype=np.float32)
    for c in range(NCORES):
        d, g = divmod(c, NGROUP)
        ysc = np.asarray(per_core_ys[c])  # [nblocks, 128, 128, MC, BL] fp16
        # out[b, 128j+tl, 128m+p] = ys[j, p, tl, m, b]
        y = ysc.transpose(4, 0, 2, 3, 1).reshape(BL, steps, H).astype(np.float32)
        out[g * BL : (g + 1) * BL, :, d * H : (d + 1) * H] = y
    return out


def kernel(**inputs):
    nc = get_program(T)
    in_maps = make_in_maps(
        inputs["x"], inputs["Wf"], inputs["Uf"], inputs["bf"],
        inputs["Wb"], inputs["Ub"], inputs["bb"],
    )
    from concourse.bass_utils import run_bass_kernel_spmd

    res = run_bass_kernel_spmd(nc, in_maps, list(range(NCORES)))
    return assemble_output([res.results[c]["ys"] for c in range(NCORES)])
